# revision 7
# baseline (speedup 1.0000x reference)
"""FFM CrossLayer pairwise-interaction kernel for 8x Trainium2 NeuronCores.

Math: out[b] = sum_{i<j} <K[i,f_j,:], K[j,f_i,:]> * x[b,i] * x[b,j]
With W[i,j] = sum_o K[i,f_j,o]*K[j,f_i,o] (symmetric), this equals
    out[b] = 0.5 * (x_b^T W x_b - sum_i W[i,i] * x[b,i]^2).

Strategy (v2, stripe-sharded, no W collective):
  Core c owns the column stripe J_c = [64c, 64c+64) of W.
  Phase A (local): W[:, J_c] via
      term1[i,(j,o)] = K[i, f_j, o]   (gpsimd ap_gather from G=[D, F*O] bf16)
      term2[i,(j,o)] = K[j, f_i, o]   (PE one-hot matmul: E_if^T @ R_c)
      W_ib = reduce_o(term1 * term2)  (scalar psum->bf16 copy, DVE mul +
                                       grouped reduce), [128, 64] bf16 per
                                       i-block; cols 64..127 of the lhsT get
                                       W_ib * dmask_ib (diag correction).
  Phase B (full batch per core): per 512-batch chunk,
      psum[0:64,  b] = sum_i W[i, j] xT[i, b]      (main term)
      psum[64:128,b] = -W[jj, jj] xT[jj, b]        (diag term, via dmask cols)
      z = psum * xs2 (x stripe stacked twice), out_part = matmul(0.5-ones, z).
  Collective: ReduceScatter(add) of the 8 per-core [4096] partials -> [512].
"""

import sys

import numpy as np

try:  # the grading env may or may not have concourse on sys.path already
    import concourse.bass as bass  # noqa: F401
except ImportError:
    sys.path.insert(0, "/opt/trn_rl_repo")

import ml_dtypes

import concourse.bacc as bacc
import concourse.bass as bass
import concourse.mybir as mybir
import concourse.tile as tile
from concourse.bass_utils import run_bass_kernel_spmd

B, D, F, O = 4096, 512, 64, 64
NC = 8            # cores
JS = D // NC      # stripe width (64)
P = 128           # partitions
NIB = D // P      # i blocks (4)
NBC = B // 512    # batch chunks (8)
F32 = mybir.dt.float32
BF16 = mybir.dt.bfloat16
NPBF16 = ml_dtypes.bfloat16

_CACHE = {}


def _build_program():
    nc = bacc.Bacc("TRN2", target_bir_lowering=False, debug=False, num_devices=NC)

    g = nc.dram_tensor("g", [D, F * O], BF16, kind="ExternalInput").ap()
    rc = nc.dram_tensor("rc", [F, JS * O], BF16, kind="ExternalInput").ap()
    eif = nc.dram_tensor("eif", [F, D], BF16, kind="ExternalInput").ap()
    idx = nc.dram_tensor("idx", [P, JS // 16], mybir.dt.int16, kind="ExternalInput").ap()
    xt = nc.dram_tensor("xt", [D, B], BF16, kind="ExternalInput").ap()
    xs2 = nc.dram_tensor("xs2", [P, B], BF16, kind="ExternalInput").ap()
    dmask = nc.dram_tensor("dmask", [D, JS], BF16, kind="ExternalInput").ap()
    hov = nc.dram_tensor("hov", [P, 1], BF16, kind="ExternalInput").ap()
    outv = nc.dram_tensor("outv", [B // NC], F32, kind="ExternalOutput").ap()

    FO = F * O  # 4096

    with tile.TileContext(nc) as tc:
        with (
            tc.tile_pool(name="cst", bufs=1) as cst,
            tc.tile_pool(name="sbA", bufs=2) as sbA,
            tc.tile_pool(name="sbB", bufs=2) as sbB,
            tc.tile_pool(name="wp", bufs=1) as wp,
            tc.tile_pool(name="psA", bufs=2, space="PSUM") as psA,
            tc.tile_pool(name="psB", bufs=2, space="PSUM") as psB,
            tc.tile_pool(name="psO", bufs=2, space="PSUM") as psO,
            tc.tile_pool(name="dram", bufs=1, space="DRAM") as dram,
        ):
            # ---- constant loads: phase-A-critical first, per-block DMAs so
            # compute starts as soon as the first blocks land; issue spread
            # across engine queues to avoid serializing on Sync.
            idx_sb = cst.tile([P, JS // 16], mybir.dt.int16)
            nc.sync.dma_start(idx_sb[:], idx[:])
            rc_sb = cst.tile([F, JS * O], BF16)
            nc.sync.dma_start(rc_sb[:], rc[:])
            eif_sb = cst.tile([F, D], BF16)
            nc.sync.dma_start(eif_sb[:], eif[:])
            g_sb = []
            for ib in range(NIB):
                g_t = cst.tile([P, FO], BF16, tag=f"g{ib}")
                nc.sync.dma_start(g_t[:], g[ib * P : (ib + 1) * P, :])
                g_sb.append(g_t)
            # phase B deps on other queues
            dm_sb = cst.tile([P, NIB * JS], BF16)
            nc.scalar.dma_start(
                dm_sb[:].rearrange("p (a j) -> p a j", a=NIB),
                dmask[:].rearrange("(a p) j -> p a j", p=P),
            )
            hov_sb = cst.tile([P, 1], BF16)
            nc.scalar.dma_start(hov_sb[:], hov[:])
            xs2_sb = cst.tile([P, B], BF16)
            nc.scalar.dma_start(xs2_sb[:], xs2[:])
            xt_sb = []
            for ib in range(NIB):
                xt_t = cst.tile([P, B], BF16, tag=f"xt{ib}")
                nc.gpsimd.dma_start(xt_t[:], xt[ib * P : (ib + 1) * P, :])
                xt_sb.append(xt_t)

            ovrow = cst.tile([1, B], F32)
            rsin = dram.tile([B], F32)
            rsout = dram.tile([B // NC], F32)

            wlhs = []
            for ib in range(NIB):
                w_t = wp.tile([P, P], BF16, tag=f"wl{ib}")
                wlhs.append(w_t)

            # ---- phase A: W[:, J_c] stripe, one i-block at a time ----
            with nc.allow_low_precision(reason="bf16 W stripe; tol 2e-2"):
                for ib in range(NIB):
                    t1 = sbA.tile([P, FO], BF16, tag="t1")
                    nc.gpsimd.ap_gather(
                        t1[:], g_sb[ib][:], idx_sb[:],
                        channels=P, num_elems=F, d=O, num_idxs=JS,
                    )
                    for q in range(4):  # 1024 cols = 16 j x 64 o
                        ps = psA.tile([P, 1024], F32, tag="psA")
                        for n in range(2):
                            nc.tensor.matmul(
                                ps[:, n * 512 : (n + 1) * 512],
                                eif_sb[:, ib * P : (ib + 1) * P],
                                rc_sb[:, q * 1024 + n * 512 : q * 1024 + (n + 1) * 512],
                                start=True, stop=True,
                            )
                        t2b = sbA.tile([P, 1024], BF16, tag="t2b")
                        nc.scalar.copy(t2b[:], ps[:])
                        zb = sbA.tile([P, 1024], BF16, tag="zb")
                        nc.vector.tensor_mul(
                            zb[:], t1[:, q * 1024 : (q + 1) * 1024], t2b[:]
                        )
                        nc.vector.tensor_reduce(
                            wlhs[ib][:, q * 16 : q * 16 + 16],
                            zb[:].rearrange("p (j o) -> p j o", o=O),
                            axis=mybir.AxisListType.X, op=mybir.AluOpType.add,
                        )
                    # diag-correction columns 64..127 of the lhsT
                    nc.vector.tensor_mul(
                        wlhs[ib][:, JS : 2 * JS],
                        wlhs[ib][:, 0:JS],
                        dm_sb[:, ib * JS : (ib + 1) * JS],
                    )

            # ---- phase B: y^T = lhsT^T @ xT over full batch, fused epilogue ----
            for bc in range(NBC):
                yp = psB.tile([P, 512], F32, tag="yp")
                for ib in range(NIB):
                    nc.tensor.matmul(
                        yp[:],
                        wlhs[ib][:],
                        xt_sb[ib][:, bc * 512 : (bc + 1) * 512],
                        start=(ib == 0), stop=(ib == NIB - 1),
                    )
                z = sbB.tile([P, 512], BF16, tag="z")
                nc.vector.tensor_mul(z[:], yp[:], xs2_sb[:, bc * 512 : (bc + 1) * 512])
                op = psO.tile([1, 512], F32, tag="op")
                nc.tensor.matmul(op[:], hov_sb[:], z[:], start=True, stop=True)
                nc.scalar.copy(ovrow[:, bc * 512 : (bc + 1) * 512], op[:])

            # ---- collective: tiny ReduceScatter of the partial outputs ----
            nc.sync.dma_start(rsin[:], ovrow[:])
            nc.gpsimd.collective_compute(
                "ReduceScatter", mybir.AluOpType.add,
                replica_groups=[list(range(NC))],
                ins=[rsin.opt()], outs=[rsout.opt()],
            )
            nc.sync.dma_start(outv[:], rsout[:])

    nc.compile()
    return nc


def _host_prep(x, kern, field_ids):
    x = np.asarray(x, dtype=np.float32)
    k = np.asarray(kern, dtype=np.float32)
    fid = np.asarray(field_ids).astype(np.int64).ravel()
    assert x.shape == (B, D) and k.shape == (D, F, O) and fid.shape == (D,)

    g = np.ascontiguousarray(k.reshape(D, F * O)).astype(NPBF16)
    eif = (fid[None, :] == np.arange(F)[:, None]).astype(NPBF16)  # [F, D]
    xt = np.ascontiguousarray(x.T).astype(NPBF16)                  # [D, B]
    hov = np.full((P, 1), 0.5, NPBF16)

    in_maps = []
    for c in range(NC):
        jlo = c * JS
        jc = slice(jlo, jlo + JS)
        rc = np.ascontiguousarray(k[jc].transpose(1, 0, 2).reshape(F, JS * O)).astype(NPBF16)
        fj = fid[jc].astype(np.int16)
        idx16 = np.zeros((16, JS // 16), np.int16)
        for t in range(JS):
            idx16[t % 16, t // 16] = fj[t]
        idx = np.tile(idx16, (P // 16, 1))
        xsl = xt[jc]                                   # [64, B] bf16
        xs2 = np.concatenate([xsl, xsl], axis=0)       # [128, B]
        dmask = np.zeros((D, JS), NPBF16)
        for t in range(JS):
            dmask[jlo + t, t] = -1.0
        in_maps.append({
            "g": g, "rc": rc, "eif": eif, "idx": idx,
            "xt": xt, "xs2": xs2, "dmask": dmask, "hov": hov,
        })
    return in_maps


def kernel(x, kernel, field_ids):
    if "nc" not in _CACHE:
        _CACHE["nc"] = _build_program()
    nc = _CACHE["nc"]
    in_maps = _host_prep(x, kernel, field_ids)
    res = run_bass_kernel_spmd(nc, in_maps, core_ids=list(range(NC)))
    out = np.concatenate([np.asarray(res.results[c]["outv"]).ravel() for c in range(NC)])
    return out.astype(np.float32)


# revision 10
# speedup vs baseline: 1.2121x; 1.2121x over previous
"""FFM CrossLayer pairwise-interaction kernel for 8x Trainium2 NeuronCores.

Math: out[b] = sum_{i<j} <K[i,f_j,:], K[j,f_i,:]> * x[b,i] * x[b,j]
With W[i,j] = sum_o K[i,f_j,o]*K[j,f_i,o] (symmetric), this equals
    out[b] = 0.5 * (x_b^T W x_b - sum_i W[i,i] * x[b,i]^2).

Strategy (v2, stripe-sharded, no W collective):
  Core c owns the column stripe J_c = [64c, 64c+64) of W.
  Phase A (local): W[:, J_c] via
      term1[i,(j,o)] = K[i, f_j, o]   (gpsimd ap_gather from G=[D, F*O] bf16)
      term2[i,(j,o)] = K[j, f_i, o]   (PE one-hot matmul: E_if^T @ R_c)
      W_ib = reduce_o(term1 * term2)  (scalar psum->bf16 copy, DVE mul +
                                       grouped reduce), [128, 64] bf16 per
                                       i-block; cols 64..127 of the lhsT get
                                       W_ib * dmask_ib (diag correction).
  Phase B (full batch per core): per 512-batch chunk,
      psum[0:64,  b] = sum_i W[i, j] xT[i, b]      (main term)
      psum[64:128,b] = -W[jj, jj] xT[jj, b]        (diag term, via dmask cols)
      z = psum * xs2 (x stripe stacked twice), out_part = matmul(0.5-ones, z).
  Collective: ReduceScatter(add) of the 8 per-core [4096] partials -> [512].
"""

import sys

import numpy as np

try:  # the grading env may or may not have concourse on sys.path already
    import concourse.bass as bass  # noqa: F401
except ImportError:
    sys.path.insert(0, "/opt/trn_rl_repo")

import ml_dtypes

import concourse.bacc as bacc
import concourse.bass as bass
import concourse.mybir as mybir
import concourse.tile as tile
from concourse.bass_utils import run_bass_kernel_spmd

B, D, F, O = 4096, 512, 64, 64
NC = 8            # cores
JS = D // NC      # stripe width (64)
P = 128           # partitions
NIB = D // P      # i blocks (4)
NBC = B // 512    # batch chunks (8)
F32 = mybir.dt.float32
BF16 = mybir.dt.bfloat16
NPBF16 = ml_dtypes.bfloat16

_CACHE = {}


def _build_program():
    nc = bacc.Bacc("TRN2", target_bir_lowering=False, debug=False, num_devices=NC)

    g = nc.dram_tensor("g", [D, F * O], BF16, kind="ExternalInput").ap()
    rc = nc.dram_tensor("rc", [F, JS * O], BF16, kind="ExternalInput").ap()
    eif = nc.dram_tensor("eif", [F, D], BF16, kind="ExternalInput").ap()
    idx = nc.dram_tensor("idx", [P, JS // 16], mybir.dt.int16, kind="ExternalInput").ap()
    xt = nc.dram_tensor("xt", [D, B], BF16, kind="ExternalInput").ap()
    xs2 = nc.dram_tensor("xs2", [P, B], BF16, kind="ExternalInput").ap()
    dmask = nc.dram_tensor("dmask", [D, JS], BF16, kind="ExternalInput").ap()
    hov = nc.dram_tensor("hov", [P, 1], BF16, kind="ExternalInput").ap()
    outv = nc.dram_tensor("outv", [B // NC], F32, kind="ExternalOutput").ap()

    FO = F * O  # 4096

    with tile.TileContext(nc) as tc:
        with (
            tc.tile_pool(name="cst", bufs=1) as cst,
            tc.tile_pool(name="sbA", bufs=2) as sbA,
            tc.tile_pool(name="sbB", bufs=2) as sbB,
            tc.tile_pool(name="wp", bufs=1) as wp,
            tc.tile_pool(name="psA", bufs=2, space="PSUM") as psA,
            tc.tile_pool(name="psB", bufs=2, space="PSUM") as psB,
            tc.tile_pool(name="psO", bufs=2, space="PSUM") as psO,
            tc.tile_pool(name="dram", bufs=1, space="DRAM") as dram,
        ):
            # ---- constant loads: phase-A-critical first, per-block DMAs so
            # compute starts as soon as the first blocks land; issue spread
            # across engine queues to avoid serializing on Sync.
            # sync ring: gather-critical loads lead (idx, then g blocks in
            # consumption order); small phase-B tensors after; xs2 last.
            idx_sb = cst.tile([P, JS // 16], mybir.dt.int16)
            nc.sync.dma_start(idx_sb[:], idx[:])
            g_sb = []
            for ib in range(NIB):
                g_t = cst.tile([P, FO], BF16, tag=f"g{ib}")
                nc.sync.dma_start(g_t[:], g[ib * P : (ib + 1) * P, :])
                g_sb.append(g_t)
            dm_sb = cst.tile([P, NIB * JS], BF16)
            nc.sync.dma_start(
                dm_sb[:].rearrange("p (a j) -> p a j", a=NIB),
                dmask[:].rearrange("(a p) j -> p a j", p=P),
            )
            hov_sb = cst.tile([P, 1], BF16)
            nc.sync.dma_start(hov_sb[:], hov[:])
            xs2_sb = cst.tile([P, B], BF16)
            nc.sync.dma_start(xs2_sb[:], xs2[:])
            # scalar ring: matmul-critical loads (needed ~when the first
            # term2 matmuls fire), kept off the g ring so both stream.
            rc_sb = cst.tile([F, JS * O], BF16)
            nc.scalar.dma_start(rc_sb[:], rc[:])
            eif_sb = cst.tile([F, D], BF16)
            nc.scalar.dma_start(eif_sb[:], eif[:])
            # xt issues on the gpsimd queue AFTER gather 0 (see phase A) so
            # its 4MB never races the g blocks into the rings.
            xt_sb = []
            for ib in range(NIB):
                xt_t = cst.tile([P, B], BF16, tag=f"xt{ib}")
                xt_sb.append(xt_t)

            ovrow = cst.tile([1, B], F32)
            rsin = dram.tile([B], F32)
            rsout = dram.tile([B // NC], F32)

            wlhs = []
            for ib in range(NIB):
                w_t = wp.tile([P, P], BF16, tag=f"wl{ib}")
                wlhs.append(w_t)

            # ---- phase A: W[:, J_c] stripe, one i-block at a time ----
            with nc.allow_low_precision(reason="bf16 W stripe; tol 2e-2"):
                for ib in range(NIB):
                    t1 = sbA.tile([P, FO], BF16, tag="t1")
                    nc.gpsimd.ap_gather(
                        t1[:], g_sb[ib][:], idx_sb[:],
                        channels=P, num_elems=F, d=O, num_idxs=JS,
                    )
                    if ib == 0:
                        # xt stream starts now: behind gather0 on the gpsimd
                        # queue, so the g blocks won the rings first.
                        for jb in range(NIB):
                            nc.gpsimd.dma_start(
                                xt_sb[jb][:], xt[jb * P : (jb + 1) * P, :]
                            )
                    for q in range(4):  # 1024 cols = 16 j x 64 o
                        ps = psA.tile([P, 1024], F32, tag="psA")
                        for n in range(2):
                            nc.tensor.matmul(
                                ps[:, n * 512 : (n + 1) * 512],
                                eif_sb[:, ib * P : (ib + 1) * P],
                                rc_sb[:, q * 1024 + n * 512 : q * 1024 + (n + 1) * 512],
                                start=True, stop=True,
                            )
                        t2b = sbA.tile([P, 1024], BF16, tag="t2b")
                        nc.scalar.copy(t2b[:], ps[:])
                        zb = sbA.tile([P, 1024], BF16, tag="zb")
                        nc.vector.tensor_mul(
                            zb[:], t1[:, q * 1024 : (q + 1) * 1024], t2b[:]
                        )
                        nc.vector.tensor_reduce(
                            wlhs[ib][:, q * 16 : q * 16 + 16],
                            zb[:].rearrange("p (j o) -> p j o", o=O),
                            axis=mybir.AxisListType.X, op=mybir.AluOpType.add,
                        )
                    # diag-correction columns 64..127 of the lhsT
                    nc.vector.tensor_mul(
                        wlhs[ib][:, JS : 2 * JS],
                        wlhs[ib][:, 0:JS],
                        dm_sb[:, ib * JS : (ib + 1) * JS],
                    )

            # ---- phase B: y^T = lhsT^T @ xT over full batch, fused epilogue ----
            for bc in range(NBC):
                yp = psB.tile([P, 512], F32, tag="yp")
                for ib in range(NIB):
                    nc.tensor.matmul(
                        yp[:],
                        wlhs[ib][:],
                        xt_sb[ib][:, bc * 512 : (bc + 1) * 512],
                        start=(ib == 0), stop=(ib == NIB - 1),
                    )
                z = sbB.tile([P, 512], BF16, tag="z")
                nc.vector.tensor_mul(z[:], yp[:], xs2_sb[:, bc * 512 : (bc + 1) * 512])
                op = psO.tile([1, 512], F32, tag="op")
                nc.tensor.matmul(op[:], hov_sb[:], z[:], start=True, stop=True)
                nc.scalar.copy(ovrow[:, bc * 512 : (bc + 1) * 512], op[:])

            # ---- collective: tiny ReduceScatter of the partial outputs ----
            nc.sync.dma_start(rsin[:], ovrow[:])
            nc.gpsimd.collective_compute(
                "ReduceScatter", mybir.AluOpType.add,
                replica_groups=[list(range(NC))],
                ins=[rsin.opt()], outs=[rsout.opt()],
            )
            nc.sync.dma_start(outv[:], rsout[:])

    nc.compile()
    return nc


def _host_prep(x, kern, field_ids):
    x = np.asarray(x, dtype=np.float32)
    k = np.asarray(kern, dtype=np.float32)
    fid = np.asarray(field_ids).astype(np.int64).ravel()
    assert x.shape == (B, D) and k.shape == (D, F, O) and fid.shape == (D,)

    g = np.ascontiguousarray(k.reshape(D, F * O)).astype(NPBF16)
    eif = (fid[None, :] == np.arange(F)[:, None]).astype(NPBF16)  # [F, D]
    xt = np.ascontiguousarray(x.T).astype(NPBF16)                  # [D, B]
    hov = np.full((P, 1), 0.5, NPBF16)

    in_maps = []
    for c in range(NC):
        jlo = c * JS
        jc = slice(jlo, jlo + JS)
        rc = np.ascontiguousarray(k[jc].transpose(1, 0, 2).reshape(F, JS * O)).astype(NPBF16)
        fj = fid[jc].astype(np.int16)
        idx16 = np.zeros((16, JS // 16), np.int16)
        for t in range(JS):
            idx16[t % 16, t // 16] = fj[t]
        idx = np.tile(idx16, (P // 16, 1))
        xsl = xt[jc]                                   # [64, B] bf16
        xs2 = np.concatenate([xsl, xsl], axis=0)       # [128, B]
        dmask = np.zeros((D, JS), NPBF16)
        for t in range(JS):
            dmask[jlo + t, t] = -1.0
        in_maps.append({
            "g": g, "rc": rc, "eif": eif, "idx": idx,
            "xt": xt, "xs2": xs2, "dmask": dmask, "hov": hov,
        })
    return in_maps


def kernel(x, kernel, field_ids):
    if "nc" not in _CACHE:
        _CACHE["nc"] = _build_program()
    nc = _CACHE["nc"]
    in_maps = _host_prep(x, kernel, field_ids)
    res = run_bass_kernel_spmd(nc, in_maps, core_ids=list(range(NC)))
    out = np.concatenate([np.asarray(res.results[c]["outv"]).ravel() for c in range(NC)])
    return out.astype(np.float32)
